# revision 9
# baseline (speedup 1.0000x reference)
"""Trainium2 Bass kernel for the SmoothAP + relaxed-KD criterion.

Math notes (vs the reference):
  * Only the diagonal class blocks of rk_all are consumed by the loss
    (rk4[idx, :, idx, :]), so per query row i we only need ranks against
    its 4 same-class columns -> 96x less sigmoid work than the [B,B,B]
    formulation.
  * sigmoid(clip(t, +-50)) with t = d/0.01  ==  0.5 + 0.5*tanh(50*d)
    (saturation is identical in fp32), which keeps the whole device
    kernel inside the single "exp_and_others" ACT table set (tanh + exp).
  * rk_diag[i,a] = 192.5 + 0.5 * sum_k tanh(50*sim[i,k] - 50*d4[i,a])
    rk_pos [i,a] =   2.5 + 0.5 * sum_k tanh(50*d4[i,k] - 50*d4[i,a])
    with d4[i,a] = sim[i, 4*(i//4)+a].
  * KD row terms: with x = tsim*scale/TAU, y = sim*scale/TAU,
    kl[i] = (sum_j e^{x_ij} (x_ij - y_ij))/sx_i + ln(sy_i) - ln(sx_i),
    sx = sum_j e^{x_ij}, sy = sum_j e^{y_ij}.  Device returns per-row
    (rank_row, u, sx, sy); the final ln / divide / scalar reduction over
    384 rows happens on host (that is the gather step).

Sharding: data-parallel over the 384 query rows, 48 rows per core on 8
NeuronCores.  Each core gets the full (host-pre-transposed) embedding
matrices for the key side plus its own 48-row slices, so the device never
transposes anything.
"""

import numpy as np

import concourse.bass as bass
from concourse import bacc, tile, mybir
from concourse.bass_utils import run_bass_kernel_spmd

# Problem constants (hardcoded per the harness contract).
B = 384
D = 512
SPC = 4
NCORES = 8
R = B // NCORES            # 48 query rows per core
KC = D // 128              # 4 contraction chunks
TAU = 3.0
BETA = 0.5
ALPHA = 1.0
N_EPOCHS = 100

F32 = mybir.dt.float32
AF = mybir.ActivationFunctionType
OP = mybir.AluOpType

_CACHE = {}


def _build():
    nc = bacc.Bacc("TRN2", target_bir_lowering=False, debug=False,
                   enable_asserts=False, num_devices=NCORES)

    bT_d = nc.dram_tensor("bT", [D, B], F32, kind="ExternalInput")
    tT_d = nc.dram_tensor("tT", [D, B], F32, kind="ExternalInput")
    mbT_d = nc.dram_tensor("mbT", [D, R], F32, kind="ExternalInput")
    mtT_d = nc.dram_tensor("mtT", [D, R], F32, kind="ExternalInput")
    sc_d = nc.dram_tensor("sc", [R, B], F32, kind="ExternalInput")   # scale/TAU rows
    mk_d = nc.dram_tensor("mk", [R, B], F32, kind="ExternalInput")   # same-class 0/1 rows
    out_d = nc.dram_tensor("out", [R, 4], F32, kind="ExternalOutput")

    with tile.TileContext(nc) as tc:
        with (
            tc.tile_pool(name="main", bufs=1) as pool,
            tc.tile_pool(name="psum", bufs=1, space=bass.MemorySpace.PSUM) as psum,
        ):
            # ---- loads (keys stay [d_chunk partitions, chunk, col]) ----
            bT = pool.tile([128, KC, B], F32, tag="bT")
            tT = pool.tile([128, KC, B], F32, tag="tT")
            mbT = pool.tile([128, KC, R], F32, tag="mbT")
            mtT = pool.tile([128, KC, R], F32, tag="mtT")
            sc = pool.tile([R, B], F32, tag="sc")
            mk = pool.tile([R, B], F32, tag="mk")

            nc.sync.dma_start(mbT[:], mbT_d.ap().rearrange("(c p) i -> p c i", p=128))
            nc.sync.dma_start(bT[:], bT_d.ap().rearrange("(c p) k -> p c k", p=128))
            nc.sync.dma_start(mtT[:], mtT_d.ap().rearrange("(c p) i -> p c i", p=128))
            nc.sync.dma_start(tT[:], tT_d.ap().rearrange("(c p) k -> p c k", p=128))
            nc.sync.dma_start(mk[:], mk_d.ap())
            nc.sync.dma_start(sc[:], sc_d.ap())

            # ---- similarity rows: sim[i,k] = <b_i, b_k>, my 48 rows ----
            sim_ps = psum.tile([R, B], F32, tag="sim_ps")
            tsim_ps = psum.tile([R, B], F32, tag="tsim_ps")
            for c in range(KC):
                nc.tensor.matmul(sim_ps[:], mbT[:, c, :], bT[:, c, :],
                                 start=(c == 0), stop=(c == KC - 1))
            for c in range(KC):
                nc.tensor.matmul(tsim_ps[:], mtT[:, c, :], tT[:, c, :],
                                 start=(c == 0), stop=(c == KC - 1))
            sim = pool.tile([R, B], F32, tag="sim")
            tsim = pool.tile([R, B], F32, tag="tsim")
            nc.vector.tensor_copy(sim[:], sim_ps[:])
            nc.vector.tensor_copy(tsim[:], tsim_ps[:])

            out_t = pool.tile([R, 4], F32, tag="out")
            zb = pool.tile([R, 1], F32, tag="zb")
            nc.vector.memset(zb[:], 0.0)

            # ---- KD branch ----
            y = pool.tile([R, B], F32, tag="y")
            x = pool.tile([R, B], F32, tag="x")
            nc.vector.tensor_mul(y[:], sim[:], sc[:])
            nc.vector.tensor_mul(x[:], tsim[:], sc[:])
            ey = pool.tile([R, B], F32, tag="ey")
            ex = pool.tile([R, B], F32, tag="ex")
            # sy -> out col 3, sx -> out col 2
            nc.scalar.activation(ey[:], y[:], AF.Exp, bias=zb[:], scale=1.0,
                                 accum_out=out_t[:, 3:4])
            nc.scalar.activation(ex[:], x[:], AF.Exp, bias=zb[:], scale=1.0,
                                 accum_out=out_t[:, 2:3])
            dxy = pool.tile([R, B], F32, tag="dxy")
            nc.vector.tensor_sub(dxy[:], x[:], y[:])
            w = pool.tile([R, B], F32, tag="w")
            # u = sum_j ex * (x - y) -> out col 1
            nc.vector.scalar_tensor_tensor(
                out=w[:], in0=ex[:], scalar=1.0, in1=dxy[:],
                op0=OP.mult, op1=OP.mult, accum_out=out_t[:, 1:2])

            # ---- rank branch ----
            masked = pool.tile([R, B], F32, tag="masked")
            nc.vector.tensor_mul(masked[:], sim[:], mk[:])
            d4 = pool.tile([R, SPC], F32, tag="d4")
            # d4[i,a] = sum_b masked[i, 4b+a]  (one nonzero block)
            nc.vector.reduce_sum(
                d4[:], masked[:].rearrange("p (b a) -> p a b", a=SPC),
                axis=mybir.AxisListType.X)
            nd4 = pool.tile([R, SPC], F32, tag="nd4")
            nc.vector.tensor_scalar_mul(nd4[:], d4[:], -50.0)

            S = pool.tile([R, SPC], F32, tag="S")
            T4 = pool.tile([R, SPC], F32, tag="T4")
            tnh = pool.tile([R, B], F32, tag="tnh")
            tnh4 = pool.tile([R, SPC], F32, tag="tnh4")
            for a in range(SPC):
                nc.scalar.activation(tnh[:], sim[:], AF.Tanh,
                                     bias=nd4[:, a:a + 1], scale=50.0,
                                     accum_out=S[:, a:a + 1])
                nc.scalar.activation(tnh4[:], d4[:], AF.Tanh,
                                     bias=nd4[:, a:a + 1], scale=50.0,
                                     accum_out=T4[:, a:a + 1])

            rkp = pool.tile([R, SPC], F32, tag="rkp")
            rkd = pool.tile([R, SPC], F32, tag="rkd")
            nc.vector.tensor_scalar(rkp[:], T4[:], 0.5, 2.5, OP.mult, OP.add)
            nc.vector.tensor_scalar(rkd[:], S[:], 0.5, 192.5, OP.mult, OP.add)
            rinv = pool.tile([R, SPC], F32, tag="rinv")
            nc.vector.reciprocal(rinv[:], rkd[:])
            ratio = pool.tile([R, SPC], F32, tag="ratio")
            # rank_row = sum_a rkp/rkd -> out col 0
            nc.vector.scalar_tensor_tensor(
                out=ratio[:], in0=rkp[:], scalar=1.0, in1=rinv[:],
                op0=OP.mult, op1=OP.mult, accum_out=out_t[:, 0:1])

            nc.sync.dma_start(out_d.ap(), out_t[:])

    nc.compile()
    return nc


def _get_nc():
    if "nc" not in _CACHE:
        _CACHE["nc"] = _build()
    return _CACHE["nc"]


def _io_spec(nc):
    """(in_names, out_names, out_avals, partition_name) from the BIR module."""
    import jax

    partition_name = nc.partition_id_tensor.name if nc.partition_id_tensor else None
    in_names, out_names, out_avals = [], [], []
    for alloc in nc.m.functions[0].allocations:
        if not isinstance(alloc, mybir.MemoryLocationSet):
            continue
        name = alloc.memorylocations[0].name
        if alloc.kind == "ExternalInput":
            if name != partition_name:
                in_names.append(name)
        elif alloc.kind == "ExternalOutput":
            shape = tuple(alloc.tensor_shape)
            dtype = mybir.dt.np(alloc.dtype)
            out_names.append(name)
            out_avals.append(jax.core.ShapedArray(shape, dtype))
    return in_names, out_names, out_avals, partition_name


def _make_exec(nc, chain_k=1, donate=True):
    """Compile an 8-core SPMD executable for `nc` once; returns
    (fn, in_names, out_names, out_avals).  `chain_k` chains k sequential
    NEFF executions inside the jit (for slope timing)."""
    import jax
    import numpy as _np
    from jax.experimental.shard_map import shard_map
    from jax.sharding import Mesh, PartitionSpec

    from concourse import bass2jax

    bass2jax.install_neuronx_cc_hook()
    in_names, out_names, out_avals, partition_name = _io_spec(nc)
    n_params = len(in_names)
    all_names = in_names + out_names
    if partition_name is not None:
        all_names = all_names + [partition_name]
    all_names = tuple(all_names)
    n_outs = len(out_names)

    del chain_k  # neuronx_cc_hook only supports one bass_exec per module

    def _body(*args):
        operands = list(args)
        if partition_name is not None:
            operands.append(bass2jax.partition_id_tensor())
        outs = bass2jax._bass_exec_p.bind(
            *operands,
            out_avals=tuple(out_avals),
            in_names=all_names,
            out_names=tuple(out_names),
            lowering_input_output_aliases=(),
            sim_require_finite=True,
            sim_require_nnan=True,
            nc=nc,
        )
        return tuple(outs)

    devices = jax.devices()[:NCORES]
    mesh = Mesh(_np.asarray(devices), ("core",))
    in_specs = (PartitionSpec("core"),) * (n_params + n_outs)
    out_specs = (PartitionSpec("core"),) * n_outs
    fn = jax.jit(
        shard_map(_body, mesh=mesh, in_specs=in_specs, out_specs=out_specs,
                  check_rep=False),
        donate_argnums=tuple(range(n_params, n_params + n_outs)) if donate else (),
        keep_unused=True,
    )
    return fn, in_names, out_names, out_avals


def _get_runner():
    if "runner" not in _CACHE:
        _CACHE["runner"] = _make_exec(_get_nc(), chain_k=1, donate=True)
    return _CACHE["runner"]


def _concat_inputs(in_maps, in_names):
    return [
        np.concatenate([in_maps[c][n] for c in range(NCORES)], axis=0)
        for n in in_names
    ]


def kernel(batch, teacher_batch, labels, epoch):
    batch = np.ascontiguousarray(np.asarray(batch, dtype=np.float32))
    teacher = np.ascontiguousarray(np.asarray(teacher_batch, dtype=np.float32))
    labels = np.asarray(labels)
    epoch_f = float(np.asarray(epoch))

    bT = np.ascontiguousarray(batch.T)        # [D, B]
    tT = np.ascontiguousarray(teacher.T)
    same = labels[:, None] == labels[None, :]
    sc = (np.where(same, 1.0, BETA) / TAU).astype(np.float32)   # [B, B]
    mk = same.astype(np.float32)

    in_maps = []
    for m in range(NCORES):
        sl = slice(m * R, (m + 1) * R)
        in_maps.append({
            "bT": bT,
            "tT": tT,
            "mbT": np.ascontiguousarray(bT[:, sl]),
            "mtT": np.ascontiguousarray(tT[:, sl]),
            "sc": np.ascontiguousarray(sc[sl]),
            "mk": np.ascontiguousarray(mk[sl]),
        })

    fn, in_names, out_names, out_avals = _get_runner()
    concat_in = _concat_inputs(in_maps, in_names)
    concat_zeros = [
        np.zeros((NCORES * a.shape[0], *a.shape[1:]), a.dtype) for a in out_avals
    ]
    out_arrs = fn(*concat_in, *concat_zeros)
    outs = np.asarray(out_arrs[0])  # [B, 4]

    rank_rows = outs[:, 0].astype(np.float64)
    u = outs[:, 1].astype(np.float64)
    sx = outs[:, 2].astype(np.float64)
    sy = outs[:, 3].astype(np.float64)

    loss_rank = 1.0 - rank_rows.sum() / (SPC * B)
    kl = u / sx + np.log(sy) - np.log(sx)
    loss_kd = kl.mean()
    loss = loss_rank + (epoch_f / N_EPOCHS) * ALPHA * (TAU ** 2) * loss_kd
    return (np.float32(loss), np.float32(loss_rank), np.float32(loss_kd))
